# revision 53
# baseline (speedup 1.0000x reference)
"""Trainium2 Bass kernel for BasicQuantumAttention.

Contract: kernel(**inputs) takes the FULL (unsharded) numpy inputs of the
reference problem (B=4, L=2048, D=512) and returns the full output
(out_real, out_imag), each [B, L, D] float32.

Sharding: 8 NeuronCores; core c handles batch b=c//2, query half h=c%2
(1024 queries). Key/value work is split across the core pair and
exchanged via in-kernel pairwise AllGathers, so no projection work is
duplicated.

Two data-dependent / algebraic reductions on top of that:

1. Key compaction. pad_mask masks keys with exp(-inf) -> attn weight
   exactly 0, so masked key rows contribute nothing to the output. The
   host compacts each batch's unmasked key rows into a dense list padded
   to a static capacity C (multiple of 128, >= max unmasked count; the
   program is compiled per C and cached). Pad slots reuse row 0 and are
   killed by a -30000 additive score bias (exp underflows to exactly 0),
   so they also don't disturb the softmax sums. With a ~50% random mask
   this nearly halves the k/v projections, scores, and A@V matmuls.
   (The k-projection bias is dropped outright: its score contribution
   q . b_k is constant per query, which softmax cancels exactly.)

2. Out-projection folding. attn@(v@Wo^T) == (attn@v)@Wo^T, so the host
   folds W_out into the v-projection weights (M = W_out @ Wv, one 6.4
   GFLOP f32 sgemm) and the kernel's A@V matmul directly produces the
   final output features. The output projection disappears, and the A@V
   product is computed in [query, feat] layout so the deferred softmax
   normalization (diag(1/rowsum)) and the total bias (Wo@b_v + b_out,
   constant per row since softmax rows sum to 1) fuse into the single
   DVE drain op before the store.

Phase order (every exchange covered by projection matmuls):
  k-proj -> AG(k) -> vW-proj c0 -> vW-proj c1 -> AG(v) -> q-proj
  -> scores+exp(both chunks) -> per-chunk sums -> A@V+norm.
Exchange readbacks are issued ONLY on the gpsimd queue (it is serialized
behind the collectives anyway, while a collective-gated DMA on the SP/ACT
HWDGE queues would head-of-line-block later weight-stream DMAs), and the
v exchange is a SINGLE AllGather of both feature halves so every readback
carries a true data dependency on it -- the tile scheduler reorders ready
instructions over waiting ones within a queue, and with two v collectives
it would hoist an AG(v0)-gated readback above AG(v1)'s issue slot.

Layouts (all matmuls bf16, f32 PSUM accumulation):
  - per-core inputs come transposed: xqT [3072, 1024 own query rows],
    xkT [3072, CH own compacted key rows].
  - q,k projections are computed weight-stationary into ^T layout
    [feat, row]; vW is computed into row-major [row, feat] layout.
  - scores^T [key, query] = (k^T tile).T @ q^T; the per-key pad bias and
    the 1/sqrt(D) scale fold into the ACT Exp (bias/scale).
  - attn^T tiles feed the A@V matmuls as stationary operands (one load
    per key tile serves both 512-wide feature runs).
  - softmax row sums come from a DVE accumulation + one f32 ones-matmul;
    normalization and bias are fused into the output drain.
"""

import numpy as np
import ml_dtypes

B, L, D = 4, 2048, 512
P = 128
IN_F = 6 * D          # 3072 input features of the fused projection
QK_F = 2 * D          # projected features kept: q_real block + k_real block
VW_F = 2 * D          # folded (v @ Wo^T) output features = out_real|out_imag
KT = IN_F // P        # 24 contraction tiles
NCORES = 8
HALF = L // 2         # 1024 query rows owned per core
SCALE = float(D) ** -0.5
NEG = -30000.0        # additive key mask (exp underflows to exactly 0)
GROUPS = [[0, 1], [2, 3], [4, 5], [6, 7]]

_NC_CACHE = {}


def _build_program(C, reps=1):
    import os
    import concourse.bass as bass
    import concourse.bacc as bacc
    import concourse.mybir as mybir
    import concourse.tile as tile
    from contextlib import ExitStack

    CH = C // 2           # key rows projected per core (multiple of 64)
    NKT = C // P          # key tiles total (both halves)
    # vW-projection row blocks: full 128-row tiles + optional 64-row tail
    # (C is a multiple of 128, so CH can end on a half tile)
    RB = [(j * P, P) for j in range(CH // P)]
    if CH % P:
        RB.append((CH - CH % P, CH % P))
    # column chunks for the ^T-layout k projection of width CH (PSUM banks
    # hold 512 f32, and one matmul may not cross a bank boundary)
    KCH = [(0, min(CH, 512))] + ([(512, CH)] if CH > 512 else [])

    # Timing-ablation switch: skip the pair exchanges and read back own data
    # (incorrect results, identical instruction shape otherwise).
    NO_COLL = bool(os.environ.get("KERNEL_NO_COLL"))

    f32 = mybir.dt.float32
    bf16 = mybir.dt.bfloat16
    AF = mybir.ActivationFunctionType
    ALU = mybir.AluOpType
    PSUM = bass.MemorySpace.PSUM

    nc = bacc.Bacc(
        "TRN2",
        debug=False,
        enable_asserts=False,
        target_bir_lowering=False,
        num_devices=NCORES,
    )

    xqT_d = nc.dram_tensor("xqT", [IN_F, HALF], bf16, kind="ExternalInput").ap()
    xkT_d = nc.dram_tensor("xkT", [IN_F, CH], bf16, kind="ExternalInput").ap()
    wqk_d = nc.dram_tensor("wqkT", [IN_F, QK_F], bf16, kind="ExternalInput").ap()
    wvw_d = nc.dram_tensor("wvwT", [IN_F, VW_F], bf16, kind="ExternalInput").ap()
    mb_d = nc.dram_tensor("maskb", [P, NKT], f32, kind="ExternalInput").ap()
    bqk_d = nc.dram_tensor("bqk", [P, QK_F // P], f32, kind="ExternalInput").ap()
    bto_d = nc.dram_tensor("btot", [P, VW_F], f32, kind="ExternalInput").ap()
    y_d = nc.dram_tensor("y", [HALF, VW_F], f32, kind="ExternalOutput").ap()

    k_send = nc.dram_tensor("k_send", [4 * P, CH], bf16).ap()
    k_recv = nc.dram_tensor("k_recv", [8 * P, CH], bf16).ap()
    # single combined v exchange buffer (both 512-wide feature halves):
    # one AllGather means every readback has a true data dependency on it,
    # so the tile scheduler can never hoist a readback above the collective
    # and head-of-line-block the exchange queue.
    v_send = nc.dram_tensor("v_send", [CH, VW_F], bf16).ap()
    v_recv = nc.dram_tensor("v_recv", [2 * CH, VW_F], bf16).ap()

    # Every dma_start costs ~0.65us of SP.SEQ issue time + ~0.625us of
    # serial HWDGE processing REGARDLESS of size, so bulk streams are
    # loaded in rearrange-view groups (one descriptor covers several
    # 128-row tiles; per-partition chunk sizes are unchanged, so transfer
    # efficiency is identical).
    XQG = 6               # xq tiles per group load (4 groups)
    XKG = 4               # xk tiles per group load (4 singles + 5 groups)
    VRG = 2 if NKT % 2 == 0 else (3 if NKT % 3 == 0 else 1)
    # (the "p t n" 3D views feed 2D SBUF tiles: dma_start flattens the
    # per-partition (t, n) iteration into the tile's free dim)
    xq_g = xqT_d.rearrange("(g t p) n -> g p t n", t=XQG, p=P)
    xk_r = xkT_d.rearrange("(t p) n -> t p n", p=P)
    xk_g = xkT_d.rearrange("(g t p) n -> g p t n", t=XKG, p=P)
    vrecv_g = v_recv.rearrange("(g t p) n -> g p t n", t=VRG, p=P)
    ksend_g = k_send.rearrange("(i p) n -> p i n", p=P)
    wqk_r = wqk_d.rearrange("(t p) n -> t p n", p=P)
    wvw_r = wvw_d.rearrange("(t p) n -> t p n", p=P)
    ksend_r = k_send.rearrange("(i p) n -> i p n", p=P)
    krecv_r = k_recv.rearrange("(i p) n -> i p n", p=P)

    def _emit_body(tc, ctx):
        const = ctx.enter_context(tc.tile_pool(name="const", bufs=1))
        persist = ctx.enter_context(tc.tile_pool(name="persist", bufs=1))

        # declared here, loaded after the first xk/wk tiles are queued (these
        # aren't needed until the k-proj drain / scores)
        mb = const.tile([P, NKT], f32, tag="mb")
        bqk = const.tile([P, QK_F // P], f32, tag="bqk")
        ones_c = const.tile([P, 1], f32, tag="ones_c")
        nc.vector.memset(ones_c, 1.0)
        ident1 = const.tile([1, 1], f32, tag="ident1")
        nc.vector.memset(ident1, 1.0)
        zeros = const.tile([P, 512], f32, tag="zeros")
        nc.vector.memset(zeros, 0.0)

        # Free-dim bias comes pre-broadcast from the host (loaded after the
        # critical projection streams are queued).
        bto = persist.tile([P, VW_F], f32, tag="bto")

        # Persistent attention operands.  q is split per query chunk
        # (dependency tracking is tile-granular: with one [P, HALF] tile the
        # ch0 scores would wait on the ch1 bias drains too).  v lives in
        # readback-group tiles so one grouped DMA serves several key tiles.
        q_sb = [[persist.tile([P, 512], bf16, tag=f"q{ch}_{m}", name=f"q{ch}_{m}")
                 for m in range(4)] for ch in range(2)]
        k_sb = [persist.tile([P, C], bf16, tag=f"k{m}", name=f"k{m}") for m in range(4)]
        v_gb = [persist.tile([P, VRG * VW_F], bf16, tag=f"vg{g}", name=f"vg{g}")
                for g in range(NKT // VRG)]

        def v_ap(t, cols):
            return v_gb[t // VRG][:, (t % VRG) * VW_F + cols.start:
                                  (t % VRG) * VW_F + cols.stop]

        # Single staging tile for the k^T exchange: all four feature blocks
        # go out in ONE send DMA via the (i p) n -> p (i n) dram view.
        kst = persist.tile([P, 4 * CH], bf16, tag="kst", name="kst")

        with (
            tc.tile_pool(name="xp", bufs=1) as xp,
            tc.tile_pool(name="ws", bufs=12) as ws,
            tc.tile_pool(name="vstp", bufs=2) as vstp,
        ):
            # xk: four single-tile loads for a fast k-proj start, then five
            # 4-tile grouped loads.  wk singles interleave so the weight
            # stream leads its consumption point.
            xk_s = []
            xk_grp = [xp.tile([P, XKG * CH], bf16, tag=f"xkg{g}", name=f"xkg{g}")
                      for g in range(1, KT // XKG)]

            def xk_ap(k, lo, hi):
                if k < XKG:
                    return xk_s[k][:, lo:hi]
                g, tt = divmod(k, XKG)
                return xk_grp[g - 1][:, tt * CH + lo:tt * CH + hi]

            wk = []
            for k in range(KT):
                if k < XKG:
                    xt = xp.tile([P, CH], bf16, tag=f"xk{k}")
                    nc.sync.dma_start(xt, xk_r[k])
                    xk_s.append(xt)
                elif k % XKG == 0:
                    nc.sync.dma_start(xk_grp[k // XKG - 1], xk_g[k // XKG])
                wkt = ws.tile([P, 512], bf16, tag="wst", name=f"wk{k}")
                nc.sync.dma_start(wkt, wqk_r[k, :, 512:])
                wk.append(wkt)
                if k == 1:
                    nc.sync.dma_start(bqk, bqk_d)
                    nc.sync.dma_start(mb, mb_d)
            # xq prefetch is interleaved into the SP weight streams below as
            # four 6-tile grouped loads.  It must stay on the SP queue: the
            # scheduler hoists ready DMAs over waiting ones within a queue,
            # so on any other queue these 6.3 MB would load eagerly at t=0
            # and starve the xk stream (SP's all-ready FIFO is the only
            # queue where emission position actually paces a transfer).
            xq_grp = [xp.tile([P, XQG * HALF], bf16, tag=f"xqg{g}", name=f"xqg{g}")
                      for g in range(KT // XQG)]

            def xq_ap(k, lo, hi):
                g, tt = divmod(k, XQG)
                return xq_grp[g][:, tt * HALF + lo:tt * HALF + hi]

            # ---- k_real projection (^T layout), single pass.  PSUM
            # accumulation is order-independent, so the last TAIL k-steps are
            # emitted chain-major: chain m finishes early and its ACT drain +
            # send DMA overlap the remaining matmuls instead of serializing
            # at the phase edge.
            TAIL = 3
            with tc.tile_pool(name="psk", bufs=1, space=PSUM) as psk:
                # one [P, CH] PSUM tile per chain (2 banks for CH > 512):
                # matmuls still target per-bank column chunks, but the
                # merged tile keeps bank aliasing fine-grained so the next
                # phase's PSUM tiles only wait on the chains they overlap.
                pk = [psk.tile([P, CH], f32, tag=f"pk{m}", name=f"pk{m}")
                      for m in range(4)]

                def _kmm(k, m, start, stop):
                    for (c0, c1) in KCH:
                        nc.tensor.matmul(
                            pk[m][:, c0:c1],
                            wk[k][:, m * P:(m + 1) * P], xk_ap(k, c0, c1),
                            start=start, stop=stop,
                        )

                for k in range(KT - TAIL):
                    for m in range(4):
                        _kmm(k, m, k == 0, False)
                # The k bias is dropped entirely: its score contribution
                # q . b_k is constant per query, which softmax cancels
                # exactly.  The drains are therefore plain copies, split
                # across ACT and DVE so the serial drain chain halves.
                for m in range(4):
                    for k in range(KT - TAIL, KT):
                        _kmm(k, m, False, k == KT - 1)
                    # The pool-close barrier waits for ALL chains' drains,
                    # and m=3 stops only at the final matmul — so m=3 goes
                    # to ACT (idle by then, having drained m=0 early) while
                    # m=1/m=2 share DVE mid-phase.
                    for (c0, c1) in KCH:
                        if m in (0, 3):
                            nc.scalar.activation(kst[:, m * CH + c0:m * CH + c1],
                                                 pk[m][:, c0:c1], AF.Identity)
                        else:
                            nc.vector.tensor_copy(kst[:, m * CH + c0:m * CH + c1],
                                                  pk[m][:, c0:c1])
                nc.sync.dma_start(ksend_g, kst)

            # Pairwise exchange of the k^T blocks (ranks are [even, odd] =
            # [global first half, global second half] for both pair members);
            # hidden behind the vW and q projections that follow.
            if not NO_COLL:
                nc.gpsimd.collective_compute(
                    "AllGather", mybir.AluOpType.bypass,
                    replica_groups=GROUPS,
                    ins=[k_send.opt()], outs=[k_recv.opt()],
                )

            # ---- folded (v @ Wo^T) projection, row-major, feature-half
            # outer (weights loaded once per half); each 512-feature half is
            # exchanged as soon as it's done.
            for c in range(2):
                with tc.tile_pool(name="psv", bufs=1, space=PSUM) as psv:
                    pv = [psv.tile([r, 512], f32, tag=f"pv{j}", name=f"pv{j}")
                          for j, (_, r) in enumerate(RB)]
                    wv_sl = []
                    for k in range(KT):
                        wsl = ws.tile([P, 512], bf16, tag="wst", name=f"wv{c}_{k}")
                        nc.sync.dma_start(wsl, wvw_r[k, :, c * 512:(c + 1) * 512])
                        wv_sl.append(wsl)
                        # two grouped xq loads per feature half (at k=6 and
                        # k=18, after the young wv stream has built a lead)
                        if k % 12 == 6:
                            g = c * 2 + k // 12
                            nc.sync.dma_start(xq_grp[g], xq_g[g])
                        if k >= KT - TAIL:
                            continue
                        for j, (r0, r) in enumerate(RB):
                            nc.tensor.matmul(
                                pv[j], xk_ap(k, r0, r0 + r), wsl,
                                start=(k == 0), stop=False,
                            )
                    for j, (r0, r) in enumerate(RB):
                        for k in range(KT - TAIL, KT):
                            nc.tensor.matmul(
                                pv[j], xk_ap(k, r0, r0 + r), wv_sl[k],
                                start=False, stop=(k == KT - 1),
                            )
                        vs = vstp.tile([P, 512], bf16, tag=f"vst{j}", name=f"vst{c}_{j}")
                        nc.vector.tensor_copy(vs[0:r], pv[j])
                        nc.sync.dma_start(
                            v_send[r0:r0 + r, c * 512:(c + 1) * 512], vs[0:r]
                        )
                if c == 0:
                    # k readback on the gpsimd queue while the c1 half still
                    # projects: waits only on AG(k), and the single AG(v)
                    # below isn't send-ready before these run anyway.
                    for hh in range(2):
                        for m in range(4):
                            nc.gpsimd.dma_start(
                                k_sb[m][:, hh * CH:(hh + 1) * CH],
                                ksend_r[m] if NO_COLL else krecv_r[hh * 4 + m],
                            )
                    nc.sync.dma_start(bto, bto_d)
            if not NO_COLL:
                nc.gpsimd.collective_compute(
                    "AllGather", mybir.AluOpType.bypass,
                    replica_groups=GROUPS,
                    ins=[v_send.opt()], outs=[v_recv.opt()],
                )
            # v readbacks on the gpsimd issue queue: they carry a true data
            # dependency on AG(v), so the scheduler cannot hoist them above
            # the collective; one grouped DMA per VRG key tiles.
            for g in range(NKT // VRG):
                nc.gpsimd.dma_start(v_gb[g], vrecv_g[g])

            # ---- q_real projection (^T layout), single pass, 8 PSUM banks;
            # overlaps both v exchanges.
            with tc.tile_pool(name="psq", bufs=1, space=PSUM) as psq:
                pq = [psq.tile([P, HALF], f32, tag=f"pq{m}", name=f"pq{m}")
                      for m in range(4)]
                wq_sl = []
                for k in range(KT):
                    wsl = ws.tile([P, 512], bf16, tag="wst", name=f"wq{k}")
                    nc.sync.dma_start(wsl, wqk_r[k, :, 0:512])
                    wq_sl.append(wsl)
                    if k >= KT - TAIL:
                        continue
                    for m in range(4):
                        for c in range(2):
                            nc.tensor.matmul(
                                pq[m][:, c * 512:(c + 1) * 512],
                                wsl[:, m * P:(m + 1) * P],
                                xq_ap(k, c * 512, (c + 1) * 512),
                                start=(k == 0), stop=False,
                            )
                # Staggered tails; all ch0-half bias drains are emitted
                # before any ch1-half so the first scores chain (which reads
                # every chain's ch0 half) waits only for m=3's ch0 drain —
                # the ch1 drains then overlap the ch0 scores matmuls.  Odd
                # chains drain on DVE ((pq + bias) + 0) so the two serial
                # drain chains run in parallel.
                def _qdrain(m, ch):
                    # Balance the two serial drain chains around the pool
                    # barrier: ACT (0.6us/op) takes 3 ops ending with m0ch1,
                    # DVE (0.3us/op) takes 5 ending with m3ch1, so both
                    # chains clear ~0.3-0.6us after the final matmul.
                    cs = slice(ch * 512, (ch + 1) * 512)
                    if (ch == 0 and m % 2 == 0) or (ch == 1 and m == 0):
                        nc.scalar.activation(q_sb[ch][m], pq[m][:, cs],
                                             AF.Identity, bias=bqk[:, m:m + 1])
                    else:
                        nc.vector.scalar_tensor_tensor(
                            q_sb[ch][m], pq[m][:, cs], bqk[:, m:m + 1],
                            zeros, op0=ALU.add, op1=ALU.add,
                        )

                for m in range(4):
                    for k in range(KT - TAIL, KT):
                        for c in range(2):
                            nc.tensor.matmul(
                                pq[m][:, c * 512:(c + 1) * 512],
                                wq_sl[k][:, m * P:(m + 1) * P],
                                xq_ap(k, c * 512, (c + 1) * 512),
                                start=False, stop=(k == KT - 1),
                            )
                    _qdrain(m, 0)
                for m in range(4):
                    _qdrain(m, 1)

        # --------------------------- attention ---------------------------
        with (
            tc.tile_pool(name="at", bufs=2) as atp,
            tc.tile_pool(name="ys", bufs=2) as ysp,
            tc.tile_pool(name="sm", bufs=2) as smp,
            tc.tile_pool(name="accs", bufs=2) as accp,
            tc.tile_pool(name="pssc", bufs=2, space=PSUM) as pssc,
            tc.tile_pool(name="pssum", bufs=1, space=PSUM) as pssum,
            tc.tile_pool(name="pstp", bufs=1, space=PSUM) as pstp,
            tc.tile_pool(name="psy", bufs=2, space=PSUM) as psy,
        ):
            # Both chunks' scores+exp are emitted before any A@V work:
            # guaranteed PE work that covers a late AG(v1) regardless of how
            # slow the exchange chain runs.
            at_all = []
            for ch in range(2):
                at = []
                for t in range(NKT):
                    ps = pssc.tile([P, 512], f32, tag="sc")
                    for d in range(4):
                        nc.tensor.matmul(
                            ps, k_sb[d][:, t * P:(t + 1) * P], q_sb[ch][d],
                            start=(d == 0), stop=(d == 3),
                        )
                    a = atp.tile([P, 512], bf16, tag=f"at{t}")
                    nc.scalar.activation(
                        a, ps, AF.Exp, bias=mb[:, t:t + 1], scale=SCALE
                    )
                    at.append(a)
                at_all.append(at)

            for ch in range(2):
                at = at_all[ch]

                # softmax row-sums: accumulate the at tiles on the (idle) DVE
                # and partition-reduce with a single f32 ones-matmul.
                acc = accp.tile([P, 512], f32, tag="acc")
                nc.vector.tensor_tensor(acc, at[0], at[1], op=ALU.add)
                for t in range(2, NKT):
                    nc.vector.tensor_tensor(acc, acc, at[t], op=ALU.add)
                sp = pssum.tile([1, 512], f32, tag="sum")
                nc.tensor.matmul(sp, ones_c, acc, start=True, stop=True)
                sums = smp.tile([1, 512], f32, tag="sums")
                nc.vector.tensor_copy(sums, sp)
                rc = []
                for s in range(4):
                    tp = pstp.tile([P, 1], f32, tag="tp")
                    nc.tensor.transpose(tp, sums[0:1, s * P:(s + 1) * P], ident1)
                    r = smp.tile([P, 1], f32, tag=f"rc{s}", name=f"rc{s}")
                    nc.vector.reciprocal(r, tp)
                    rc.append(r)

                # A@V directly in [query, feat] output layout; the at slice
                # is the stationary operand, shared by both 512-wide feature
                # runs (half the LDWEIGHTS).  Deferred normalization (diag
                # scaling commutes with the row sums) and the folded bias
                # fuse into the DVE drain.
                for s in range(4):
                    r0 = ch * 512 + s * P
                    pys = [psy.tile([P, 512], f32, tag=f"y{f}", name=f"py{f}")
                           for f in range(2)]
                    last = (ch == 1 and s == 3)
                    if not last:
                        # f-inner: each at-slice stationary load serves both
                        # 512-wide feature runs (half the LDWEIGHTS).  Both
                        # halves drain into ONE [P, 1024] staging tile and
                        # leave in a single store DMA.
                        for t in range(NKT):
                            for f in range(2):
                                nc.tensor.matmul(
                                    pys[f],
                                    at[t][:, s * P:(s + 1) * P],
                                    v_ap(t, slice(f * 512, (f + 1) * 512)),
                                    start=(t == 0), stop=(t == NKT - 1),
                                )
                        ysb = ysp.tile([P, VW_F], f32, tag="ysb")
                        for f in range(2):
                            nc.vector.scalar_tensor_tensor(
                                ysb[:, f * 512:(f + 1) * 512], pys[f], rc[s],
                                bto[:, f * 512:(f + 1) * 512],
                                op0=ALU.mult, op1=ALU.add,
                            )
                        nc.sync.dma_start(y_d[r0:r0 + P, :], ysb)
                    else:
                        # very last block goes f-outer with split stores so
                        # the f=0 drain+store hides behind f=1's matmuls,
                        # shortening the kernel tail.
                        for f in range(2):
                            for t in range(NKT):
                                nc.tensor.matmul(
                                    pys[f],
                                    at[t][:, s * P:(s + 1) * P],
                                    v_ap(t, slice(f * 512, (f + 1) * 512)),
                                    start=(t == 0), stop=(t == NKT - 1),
                                )
                            ysf = ysp.tile([P, 512], f32, tag=f"ysl{f}",
                                           name=f"ysl{f}")
                            nc.vector.scalar_tensor_tensor(
                                ysf, pys[f], rc[s], bto[:, f * 512:(f + 1) * 512],
                                op0=ALU.mult, op1=ALU.add,
                            )
                            nc.sync.dma_start(
                                y_d[r0:r0 + P, f * 512:(f + 1) * 512], ysf
                            )

    with tile.TileContext(nc) as tc:
        for r in range(reps):
            if r:
                tc.strict_bb_all_engine_barrier()
            with ExitStack() as ctx:
                _emit_body(tc, ctx)

    nc.compile()
    return nc


def get_nc(C, reps=1):
    key = f"nc{C}_{reps}"
    if key not in _NC_CACHE:
        _NC_CACHE[key] = _build_program(C, reps)
    return _NC_CACHE[key]


def capacity(pad_mask):
    """Static per-batch key capacity: max unmasked count, rounded up to a
    multiple of 128 (each core projects C/2 rows — a multiple of 64, which
    the vW projection handles via a half-height tail tile)."""
    counts = (~np.asarray(pad_mask).astype(bool)).sum(axis=1)
    C = max(512, -(-int(counts.max()) // 128) * 128)
    return min(C, L)


def prepare_in_maps(inputs, C=None):
    bf = ml_dtypes.bfloat16
    f32 = np.float32

    q_real = np.asarray(inputs["q_real"], f32)
    q_imag = np.asarray(inputs["q_imag"], f32)
    k_real = np.asarray(inputs["k_real"], f32)
    k_imag = np.asarray(inputs["k_imag"], f32)
    v_real = np.asarray(inputs["v_real"], f32)
    v_imag = np.asarray(inputs["v_imag"], f32)
    pad_mask = np.asarray(inputs["pad_mask"]).astype(bool)
    W_qkv = np.asarray(inputs["W_qkv"], f32)
    b_qkv = np.asarray(inputs["b_qkv"], f32)
    W_out = np.asarray(inputs["W_out"], f32)
    b_out = np.asarray(inputs["b_out"], f32)

    if C is None:
        C = capacity(pad_mask)
    CH = C // 2
    NKT = C // P

    sel_qk = np.r_[0:D, 2 * D:3 * D]          # q_real + k_real output blocks
    wqkT = np.ascontiguousarray(W_qkv[sel_qk, :].T.astype(bf))
    # fold W_out into the v projection: attn@(v@Wo^T) with v = x@Wv^T + b_v
    # gives out = attn_norm @ (x @ (Wo@Wv)^T) + (Wo@b_v + b_out).
    Wv = W_qkv[4 * D:6 * D, :]
    wvwT = np.ascontiguousarray((W_out @ Wv).T.astype(bf))
    btot = W_out @ b_qkv[4 * D:6 * D] + b_out
    btot_b = np.ascontiguousarray(np.broadcast_to(btot, (P, VW_F)).astype(f32))
    bqk = np.ascontiguousarray(b_qkv[sel_qk].reshape(QK_F // P, P).T.astype(f32))

    x = np.concatenate([q_real, q_imag, k_real, k_imag, v_real, v_imag], axis=-1)

    in_maps = []
    for c in range(NCORES):
        b, h = divmod(c, 2)
        idx = np.flatnonzero(~pad_mask[b])
        n = len(idx)
        idx_pad = np.pad(idx, (0, C - n))     # pad slots reuse row 0
        own = idx_pad[h * CH:(h + 1) * CH]
        xqT = np.ascontiguousarray(x[b][h * HALF:(h + 1) * HALF].T.astype(bf))
        xkT = np.ascontiguousarray(x[b][own].T.astype(bf))
        # -ln(4): scales the unnormalized exp weights into comfortable bf16
        # range; cancels exactly in the softmax normalization.  Pad slots
        # get -30000 -> exp == 0.
        mbias = np.where(np.arange(C) < n, f32(-1.3862944), f32(NEG))
        mbt = np.ascontiguousarray(mbias.reshape(NKT, P).T.astype(f32))
        in_maps.append({
            "xqT": xqT, "xkT": xkT, "wqkT": wqkT, "wvwT": wvwT,
            "maskb": mbt, "bqk": bqk, "btot": btot_b,
        })
    return in_maps


def assemble_outputs(results):
    out_real = np.empty((B, L, D), np.float32)
    out_imag = np.empty((B, L, D), np.float32)
    for c in range(NCORES):
        y = np.asarray(results[c]["y"], np.float32)
        b, h = divmod(c, 2)
        out_real[b, h * HALF:(h + 1) * HALF] = y[:, :D]
        out_imag[b, h * HALF:(h + 1) * HALF] = y[:, D:]
    return out_real, out_imag


def _make_executor(C, reps=1):
    """One jitted SPMD callable per (C, reps) (mirrors
    bass2jax.run_bass_via_pjrt but is built once and reused, so repeated
    runs don't recompile)."""
    import jax
    from concourse import bass2jax, mybir

    try:
        jax.config.update("jax_compilation_cache_dir", "/tmp/jax_neff_cache")
        jax.config.update("jax_persistent_cache_min_compile_time_secs", 5.0)
    except Exception:
        pass

    nc = get_nc(C, reps)
    bass2jax.install_neuronx_cc_hook()
    partition_name = nc.partition_id_tensor.name if nc.partition_id_tensor else None

    in_names, out_names, out_avals, zero_outs = [], [], [], []
    for alloc in nc.m.functions[0].allocations:
        if not isinstance(alloc, mybir.MemoryLocationSet):
            continue
        name = alloc.memorylocations[0].name
        if alloc.kind == "ExternalInput":
            if name != partition_name:
                in_names.append(name)
        elif alloc.kind == "ExternalOutput":
            out_names.append(name)
            shape = tuple(alloc.tensor_shape)
            dtype = mybir.dt.np(alloc.dtype)
            out_avals.append(jax.core.ShapedArray(shape, dtype))
            zero_outs.append((shape, dtype))
    n_params = len(in_names)
    n_outs = len(out_avals)
    all_in_names = list(in_names) + list(out_names)
    if partition_name is not None:
        all_in_names.append(partition_name)

    def _body(*args):
        operands = list(args)
        if partition_name is not None:
            operands.append(bass2jax.partition_id_tensor())
        outs = bass2jax._bass_exec_p.bind(
            *operands,
            out_avals=tuple(out_avals),
            in_names=tuple(all_in_names),
            out_names=tuple(out_names),
            lowering_input_output_aliases=(),
            sim_require_finite=True,
            sim_require_nnan=True,
            nc=nc,
        )
        return tuple(outs)

    devices = jax.devices()[:NCORES]
    assert len(devices) == NCORES
    mesh = bass2jax.Mesh(np.asarray(devices), ("core",))
    in_specs = (bass2jax.PartitionSpec("core"),) * (n_params + n_outs)
    out_specs = (bass2jax.PartitionSpec("core"),) * n_outs
    donate = tuple(range(n_params, n_params + n_outs))
    sharded = jax.jit(
        bass2jax.shard_map(
            _body, mesh=mesh, in_specs=in_specs,
            out_specs=out_specs, check_rep=False,
        ),
        donate_argnums=donate,
        keep_unused=True,
    )
    return {
        "sharded": sharded,
        "mesh": mesh,
        "in_names": in_names,
        "out_names": out_names,
        "out_avals": out_avals,
        "zero_outs": zero_outs,
    }


def get_executor(C, reps=1):
    key = f"exec{C}_{reps}"
    if key not in _NC_CACHE:
        _NC_CACHE[key] = _make_executor(C, reps)
    return _NC_CACHE[key]


def concat_inputs(in_maps, ex):
    return [
        np.concatenate([np.asarray(in_maps[c][n]) for c in range(NCORES)], axis=0)
        for n in ex["in_names"]
    ]


def make_zero_outs(ex):
    return [
        np.zeros((NCORES * s[0], *s[1:]), d) for (s, d) in ex["zero_outs"]
    ]


def execute(concat_in, ex):
    out_arrs = ex["sharded"](*concat_in, *make_zero_outs(ex))
    results = [
        {
            name: np.asarray(out_arrs[i]).reshape(
                NCORES, *ex["out_avals"][i].shape
            )[c]
            for i, name in enumerate(ex["out_names"])
        }
        for c in range(NCORES)
    ]
    return results


def run(inputs, trace=False):
    from concourse.bass_utils import run_bass_kernel_spmd

    C = capacity(inputs["pad_mask"])
    nc = get_nc(C)
    in_maps = prepare_in_maps(inputs, C)
    return run_bass_kernel_spmd(
        nc, in_maps, core_ids=list(range(NCORES)), trace=trace
    )


def kernel(**inputs):
    C = capacity(inputs["pad_mask"])
    ex = get_executor(C)
    in_maps = prepare_in_maps(inputs, C)
    results = execute(concat_inputs(in_maps, ex), ex)
    return assemble_outputs(results)
